# revision 2
# baseline (speedup 1.0000x reference)
import sys, os
sys.path.insert(0, "/opt/trn_rl_repo")
import numpy as np
import ml_dtypes
from contextlib import ExitStack

import concourse.bass as bass
import concourse.bacc as bacc
import concourse.tile as tile
from concourse import mybir
from concourse.bass_utils import run_bass_kernel_spmd

f32 = mybir.dt.float32
bf16 = mybir.dt.bfloat16
u32 = mybir.dt.uint32
AF = mybir.ActivationFunctionType
ALU = mybir.AluOpType
AX = mybir.AxisListType
bfnp = ml_dtypes.bfloat16

B, L, D, K = 16, 4096, 1024, 5
NCORES = 8
BPC = B // NCORES
LC, DC = L // 128, D // 128
NL8 = L // 512
SCALE = 1.0 / float(np.sqrt(D))

_NC_CACHE = {}


def _build_nc():
    if "nc" in _NC_CACHE:
        return _NC_CACHE["nc"]
    nc = bacc.Bacc("TRN2", target_bir_lowering=False, debug=False,
                   num_devices=NCORES)
    dI = lambda n, s, dt=bf16: nc.dram_tensor(n, s, dt, kind="ExternalInput").ap()
    hthi_d = dI("hthi", [BPC, D, L])            # H^T hi, d-major
    natcat_d = dI("natcat", [2 * BPC * L, D])   # [hi rows; lo rows], natural
    wqk_d = dI("wqk", [D, D])                   # (w_q @ w_k.T) * SCALE
    wvc_d = dI("wvc", [D, D])                   # (w_v @ w_cmp.T) * SCALE
    ws_d = dI("ws", [DC, 128, 3])               # [wsh, wsl, wsh]
    id64_d = dI("id64", [64, 64])
    id64f_d = dI("id64f", [64, 64], f32)
    i1_d = dI("i1", [1, 1])
    i1f_d = dI("i1f", [1, 1], f32)
    offs_d = dI("offs", [1, 64], f32)           # offs[0, c*8+j] = 512*c
    scr_d = nc.dram_tensor("scr", [BPC * 64, 1], f32, kind="Internal").ap()
    sl_d = nc.dram_tensor("sl", [BPC, L], bf16, kind="ExternalOutput").ap()
    el_d = nc.dram_tensor("el", [BPC, L], bf16, kind="ExternalOutput").ap()

    NQ = 4                                       # ht quarter tiles
    QL = L // NQ                                 # 1024 l per quarter

    with tile.TileContext(nc) as tc, ExitStack() as ctx:
        res = ctx.enter_context(tc.tile_pool(name="res", bufs=1))
        stg = ctx.enter_context(tc.tile_pool(name="stg", bufs=3))
        sm = ctx.enter_context(tc.tile_pool(name="sm", bufs=1))
        psc = ctx.enter_context(tc.tile_pool(name="psc", bufs=3, space="PSUM"))
        psm = ctx.enter_context(tc.tile_pool(name="psm", bufs=1, space="PSUM"))
        pss = ctx.enter_context(tc.tile_pool(name="pss", bufs=1, space="PSUM"))

        # ---- resident tiles
        wqk_sb = res.tile([128, DC, D], bf16)
        wvc_sb = res.tile([128, DC, D], bf16)
        ws_sb = res.tile([128, DC, 3], bf16)
        id64 = res.tile([64, 64], bf16)
        id64f = res.tile([64, 64], f32)
        i1 = res.tile([1, 1], bf16)
        i1f = res.tile([1, 1], f32)
        offs = res.tile([1, 64], f32)
        ht_sb = [[res.tile([128, DC, QL], bf16, tag=f"ht{b}q{q}",
                           name=f"ht{b}q{q}") for q in range(NQ)]
                 for b in range(BPC)]

        def htc(b, lc8):
            # [128, 512] rhs slice for l-chunk lc8, per dc
            q, off = lc8 * NQ // NL8, (lc8 % (NL8 // NQ)) * 512
            return lambda dc: ht_sb[b][q][:, dc, off:off + 512]

        def load_ht(b):
            for q in range(NQ):
                for dc in range(DC):
                    nc.sync.dma_start(
                        ht_sb[b][q][:, dc, :],
                        hthi_d[b, dc * 128:(dc + 1) * 128,
                               q * QL:(q + 1) * QL])

        def load_small_consts():
            for dc in range(DC):
                nc.sync.dma_start(ws_sb[:, dc, :], ws_d[dc])
            nc.sync.dma_start(id64[:], id64_d[:])
            nc.sync.dma_start(id64f[:], id64f_d[:])
            nc.sync.dma_start(i1[:], i1_d[:])
            nc.sync.dma_start(i1f[:], i1f_d[:])
            nc.sync.dma_start(offs[:], offs_d[:])

        def load_weights():
            for dc in range(DC):
                nc.scalar.dma_start(wqk_sb[:, dc, :],
                                    wqk_d[dc * 128:(dc + 1) * 128, :])
            for dc in range(DC):
                nc.scalar.dma_start(wvc_sb[:, dc, :],
                                    wvc_d[dc * 128:(dc + 1) * 128, :])

        st = [dict() for _ in range(BPC)]   # per-example cross-phase tiles

        def s1_start_logits(b):
            ci64 = sm.tile([1, 64], f32, tag=f"ci64_{b}", name=f"ci64_{b}")
            st[b]["ci64"] = ci64
            sl_sb = sm.tile([1, L], bf16, tag="osb", name="sl_sb")
            for lc8 in range(NL8):
                rhs = htc(b, lc8)
                pc = psc.tile([64, 512], f32, tag="chunk", name="pc")
                for dc in range(DC):
                    _mm = nc.tensor.matmul(pc[0:1, :], ws_sb[:, dc, 0:1],
                                           rhs(dc), start=(dc == 0),
                                           stop=(dc == DC - 1))
                    if b == 0 and lc8 == 0 and os.environ.get("TILE_FOLLOW"):
                        tile.tile_follow(_mm, log_all_deps=True)
                nc.scalar.copy(sl_sb[0:1, lc8 * 512:(lc8 + 1) * 512],
                               pc[0:1, :])
                c8v = sm.tile([1, 8], f32, tag="c8v", name="c8v")
                c8i = sm.tile([1, 8], u32, tag="c8i", name="c8i")
                nc.vector.max(c8v[:], pc[0:1, :])
                nc.vector.max_index(c8i[:], c8v[:], pc[0:1, :])
                nc.vector.tensor_copy(ci64[0:1, lc8 * 8:(lc8 + 1) * 8], c8i[:])
            nc.scalar.dma_start(sl_d[b:b + 1, :], sl_sb[:])

        def s2a_gather(b):
            # non-PE: candidate l list -> [64,1] index tiles -> row gathers
            ci64 = st[b]["ci64"]
            nc.vector.tensor_tensor(ci64[:], ci64[:], offs[:], ALU.add)
            cif_t = sm.tile([64, 1], f32, tag="cif_t", name="cif_t")
            nc.scalar.dma_start(
                scr_d[b * 64:(b + 1) * 64].rearrange("a x -> x a"),
                ci64[0:1, :])
            nc.scalar.dma_start(cif_t[:], scr_d[b * 64:(b + 1) * 64])
            candl = sm.tile([64, 1], u32, tag="candl", name="candl")
            nc.vector.tensor_scalar(candl[:], cif_t[:], float(b * L), None,
                                    ALU.add)
            candlb = sm.tile([64, 1], u32, tag="candlb", name="candlb")
            nc.vector.tensor_scalar(candlb[:], cif_t[:],
                                    float(b * L + BPC * L), None, ALU.add)
            hi64 = sm.tile([64, D], bf16, tag="hi64", name="hi64")
            lo64 = sm.tile([64, D], bf16, tag="lo64", name="lo64")
            st[b]["hi64"], st[b]["lo64"] = hi64, lo64
            nc.gpsimd.indirect_dma_start(
                out=hi64[:], out_offset=None, in_=natcat_d[:],
                in_offset=bass.IndirectOffsetOnAxis(ap=candl[:, 0:1], axis=0))
            nc.gpsimd.indirect_dma_start(
                out=lo64[:], out_offset=None, in_=natcat_d[:],
                in_offset=bass.IndirectOffsetOnAxis(ap=candlb[:, 0:1], axis=0))

        def s2b_refine(b):
            hi64, lo64 = st[b]["hi64"], st[b]["lo64"]
            phi = pss.tile([128, DC, 64], bf16, tag="smb", name="phi")
            for dc in range(DC):
                nc.tensor.transpose(phi[:, dc, :],
                                    hi64[:, dc * 128:(dc + 1) * 128], id64[:])
            hi64T = sm.tile([128, DC, 64], bf16, tag="hi64T", name="hi64T")
            st[b]["hi64T"] = hi64T
            nc.vector.tensor_copy(hi64T[:], phi[:])
            plo = pss.tile([128, DC, 64], bf16, tag="smb", name="plo")
            for dc in range(DC):
                nc.tensor.transpose(plo[:, dc, :],
                                    lo64[:, dc * 128:(dc + 1) * 128], id64[:])
            lo64T = sm.tile([128, DC, 64], bf16, tag="lo64T", name="lo64T")
            nc.scalar.copy(lo64T[:], plo[:])

            pcor = pss.tile([128, 64], f32, tag="smf", name="pcor")
            for dc in range(DC):
                nc.tensor.matmul(pcor[0:64, 0:2], hi64T[:, dc, :],
                                 ws_sb[:, dc, 0:2],
                                 start=(dc == 0), stop=(dc == DC - 1),
                                 skip_group_check=True)
                nc.tensor.matmul(pcor[0:64, 2:3], lo64T[:, dc, :],
                                 ws_sb[:, dc, 2:3],
                                 start=False, stop=(dc == DC - 1),
                                 skip_group_check=True)
            ex64p = sm.tile([64, 1], f32, tag="ex64p", name="ex64p")
            nc.vector.tensor_reduce(ex64p[:], pcor[0:64, 0:3], AX.X, ALU.add)
            pex = pss.tile([128, 64], f32, tag="smf", name="pex")
            nc.tensor.transpose(pex[0:1, 0:64], ex64p[:], id64f[:])
            ex64f = sm.tile([1, 64], f32, tag="ex64f", name="ex64f")
            nc.vector.tensor_copy(ex64f[:], pex[0:1, 0:64])
            top8v = sm.tile([1, 8], f32, tag="top8v", name="top8v")
            nc.vector.max(top8v[:], ex64f[:])
            mask64 = sm.tile([1, 64], f32, tag="mask64", name="mask64")
            nc.vector.tensor_scalar(mask64[:], ex64f[:], top8v[0:1, 4:5],
                                    None, ALU.is_ge)
            e64 = sm.tile([1, 64], f32, tag="e64", name="e64")
            nc.scalar.activation(e64[:], ex64f[:], AF.Exp)
            nc.vector.tensor_tensor(e64[:], e64[:], mask64[:], ALU.mult)
            s64 = sm.tile([1, 1], f32, tag="s64", name="s64")
            nc.vector.tensor_reduce(s64[:], e64[:], AX.X, ALU.add)
            rs64 = sm.tile([1, 1], f32, tag="rs64", name="rs64")
            nc.vector.reciprocal(rs64[:], s64[:])
            w64 = sm.tile([1, 64], f32, tag="w64", name="w64")
            nc.vector.tensor_scalar_mul(w64[:], e64[:], rs64[:])
            pw = pss.tile([128, 64], f32, tag="smf", name="pw")
            nc.tensor.transpose(pw[0:64, 0:1], w64[:], i1f[:])
            w64T = sm.tile([64, 1], f32, tag=f"w64T_{b}", name=f"w64T_{b}")
            st[b]["w64T"] = w64T
            nc.vector.tensor_copy(w64T[:], pw[0:64, 0:1])

        def s4_P(b):
            hi64T = st[b]["hi64T"]
            pP = psm.tile([64, D], f32, tag="mid", name="pP")
            for o in range(2):
                for dci in range(DC):
                    nc.tensor.matmul(pP[:, o * 512:(o + 1) * 512],
                                     hi64T[:, dci, :],
                                     wqk_sb[:, dci, o * 512:(o + 1) * 512],
                                     start=(dci == 0), stop=(dci == DC - 1),
                                     skip_group_check=True)
            P_sb = sm.tile([64, D], bf16, tag="P_sb", name="P_sb")
            nc.scalar.copy(P_sb[:], pP[:])
            pPT = pss.tile([128, DC, 64], bf16, tag="smb", name="pPT")
            for dc in range(DC):
                nc.tensor.transpose(pPT[:, dc, :],
                                    P_sb[:, dc * 128:(dc + 1) * 128], id64[:])
            PT = sm.tile([128, DC, 64], bf16, tag=f"PT_{b}", name=f"PT_{b}")
            st[b]["PT"] = PT
            nc.vector.tensor_copy(PT[:], pPT[:])

        def s5_scores(b):
            PT, w64T = st[b]["PT"], st[b]["w64T"]
            E_sb = sm.tile([64, L], bf16, tag="E_sb", name="E_sb")
            z64a = sm.tile([64, NL8], f32, tag="z64a", name="z64a")
            for lc8 in range(NL8):
                rhs = htc(b, lc8)
                psc5 = psc.tile([64, 512], f32, tag="chunk", name="psc5")
                for dc in range(DC):
                    nc.tensor.matmul(psc5[:], PT[:, dc, :], rhs(dc),
                                     start=(dc == 0), stop=(dc == DC - 1))
                nc.scalar.activation(E_sb[:, lc8 * 512:(lc8 + 1) * 512],
                                     psc5[:], AF.Exp,
                                     accum_out=z64a[:, lc8:lc8 + 1])
            z64 = sm.tile([64, 1], f32, tag="z64", name="z64")
            nc.vector.tensor_reduce(z64[:], z64a[:], AX.X, ALU.add)
            rz64 = sm.tile([64, 1], f32, tag="rz64", name="rz64")
            nc.vector.reciprocal(rz64[:], z64[:])
            c64 = sm.tile([64, 1], bf16, tag="c64", name="c64")
            nc.vector.tensor_tensor(c64[:], w64T[:], rz64[:], ALU.mult)
            pm32 = pss.tile([128, 64], f32, tag="smf", name="pm32")
            for c in range(LC):
                nc.tensor.matmul(pm32[:, c:c + 1],
                                 E_sb[:, c * 128:(c + 1) * 128], c64[:],
                                 start=True, stop=True, skip_group_check=True)
            m32 = sm.tile([128, LC], bf16, tag=f"m32_{b}", name=f"m32_{b}")
            st[b]["m32"] = m32
            nc.vector.tensor_copy(m32[:], pm32[:, 0:LC])

        def s6_amix(b):
            m32 = st[b]["m32"]
            pam = psm.tile([64, D], f32, tag="mid", name="pam")
            for lcp in range(LC // 2):
                natc = stg.tile([128, 2, D], bf16, tag="natstg", name="natc")
                nc.sync.dma_start(
                    natc[:],
                    natcat_d[b * L + lcp * 256:b * L + (lcp + 1) * 256, :]
                    .rearrange("(t p) d -> p t d", p=128))
                for t in range(2):
                    lc = lcp * 2 + t
                    for o in range(2):
                        nc.tensor.matmul(pam[0:1, o * 512:(o + 1) * 512],
                                         m32[:, lc:lc + 1],
                                         natc[:, t, o * 512:(o + 1) * 512],
                                         start=(lc == 0), stop=(lc == LC - 1),
                                         skip_group_check=True)
            amix = sm.tile([1, D], bf16, tag="amix", name="amix")
            nc.scalar.copy(amix[:], pam[0:1, :])
            pamT = pss.tile([128, DC, 64], bf16, tag="smb", name="pamT")
            for dc in range(DC):
                nc.tensor.transpose(pamT[:, dc, 0:1],
                                    amix[0:1, dc * 128:(dc + 1) * 128], i1[:])
            amixT = sm.tile([128, DC, 1], bf16, tag=f"amixT_{b}",
                            name=f"amixT_{b}")
            st[b]["amixT"] = amixT
            nc.vector.tensor_copy(amixT[:], pamT[:, :, 0:1])

        def s7_g(b):
            amixT = st[b]["amixT"]
            pg = psm.tile([64, D], f32, tag="mid", name="pg")
            for o in range(2):
                for dci in range(DC):
                    nc.tensor.matmul(pg[0:1, o * 512:(o + 1) * 512],
                                     amixT[:, dci, :],
                                     wvc_sb[:, dci, o * 512:(o + 1) * 512],
                                     start=(dci == 0), stop=(dci == DC - 1),
                                     skip_group_check=True)
            g_sb = sm.tile([1, D], bf16, tag="g_sb", name="g_sb")
            nc.scalar.copy(g_sb[:], pg[0:1, :])
            pgT = pss.tile([128, DC, 64], bf16, tag="smb", name="pgT")
            for dc in range(DC):
                nc.tensor.transpose(pgT[:, dc, 0:1],
                                    g_sb[0:1, dc * 128:(dc + 1) * 128], i1[:])
            gT = sm.tile([128, DC, 1], bf16, tag=f"gT_{b}", name=f"gT_{b}")
            st[b]["gT"] = gT
            nc.vector.tensor_copy(gT[:], pgT[:, :, 0:1])

        def s8_el(b):
            gT = st[b]["gT"]
            el_sb = sm.tile([1, L], bf16, tag="osb", name="el_sb")
            for lc8 in range(NL8):
                rhs = htc(b, lc8)
                pe = psc.tile([64, 512], f32, tag="chunk", name="pe")
                for dc in range(DC):
                    nc.tensor.matmul(pe[0:1, :], gT[:, dc, :], rhs(dc),
                                     start=(dc == 0), stop=(dc == DC - 1))
                nc.vector.tensor_copy(el_sb[0:1, lc8 * 512:(lc8 + 1) * 512],
                                      pe[0:1, :])
            nc.scalar.dma_start(el_d[b:b + 1, :], el_sb[:])

        # ---------------- schedule ----------------
        load_small_consts()
        load_ht(0)
        load_ht(1)
        load_weights()
        s1_start_logits(0)
        s2a_gather(0)
        s1_start_logits(1)
        s2b_refine(0)
        s2a_gather(1)
        s4_P(0)
        s2b_refine(1)
        s4_P(1)
        s5_scores(0)
        s5_scores(1)
        s6_amix(0)
        s6_amix(1)
        s7_g(0)
        s8_el(0)
        s7_g(1)
        s8_el(1)

    nc.compile()
    _NC_CACHE["nc"] = nc
    return nc


def _np_reference(H, attention_mask, w_start, b_start, w_q, b_q, w_k, b_k,
                  w_v, b_v, w_cmp, b_cmp):
    NEG = -1e9
    H = H.astype(np.float32)
    pad = attention_mask == 0
    sl = (H @ w_start + b_start)[..., 0]
    sl = np.where(pad, NEG, sl)
    x = sl - sl.max(-1, keepdims=True)
    e = np.exp(x); sp = e / e.sum(-1, keepdims=True)
    idx = np.argsort(-sp, axis=-1, kind="stable")[:, :K]
    tp = np.take_along_axis(sp, idx, axis=1)
    sr = np.take_along_axis(H, idx[..., None], axis=1)
    Q = sr @ w_q + b_q
    K_ = H @ w_k + b_k
    V = H @ w_v + b_v
    sc = np.einsum('bkd,bld->bkl', Q, K_) * SCALE
    sc = np.where(pad[:, None, :], NEG, sc)
    sc = sc - sc.max(-1, keepdims=True)
    a = np.exp(sc); a = a / a.sum(-1, keepdims=True)
    ctx_ = np.einsum('bkl,bld->bkd', a, V)
    tcmp = H @ w_cmp + b_cmp
    es = np.einsum('bkd,bld->bkl', ctx_, tcmp) * SCALE
    es = np.where(pad[:, None, :], NEG, es)
    w = tp / (tp.sum(-1, keepdims=True) + 1e-9)
    el = np.einsum('bk,bkl->bl', w, es)
    el = np.where(pad, NEG, el)
    return sl, el


def kernel(**inputs):
    H = np.asarray(inputs["H"], np.float32)
    mask = np.asarray(inputs["attention_mask"])
    b_start = np.asarray(inputs["b_start"], np.float32)
    biases_zero = all(np.all(np.asarray(inputs[n]) == 0)
                      for n in ["b_q", "b_k", "b_v", "b_cmp"])
    if not bool((mask == 1).all()) or not biases_zero:
        sl, el = _np_reference(**{k: np.asarray(v) for k, v in inputs.items()})
        return np.asarray(sl, np.float32), np.asarray(el, np.float32)

    w_start = np.asarray(inputs["w_start"], np.float32)
    w_q = np.asarray(inputs["w_q"], np.float32)
    w_k = np.asarray(inputs["w_k"], np.float32)
    w_v = np.asarray(inputs["w_v"], np.float32)
    w_cmp = np.asarray(inputs["w_cmp"], np.float32)

    hi = H.astype(bfnp)
    lo = (H - hi.astype(np.float32)).astype(bfnp)
    hthi = np.ascontiguousarray(hi.transpose(0, 2, 1))            # [B, D, L]
    wqk = ((w_q @ w_k.T) * SCALE).astype(bfnp)
    wvc = ((w_v @ w_cmp.T) * SCALE).astype(bfnp)
    wsh = w_start[:, 0].astype(bfnp)
    wsl = (w_start[:, 0] - wsh.astype(np.float32)).astype(bfnp)
    ws = np.stack([wsh, wsl, wsh], axis=-1).reshape(DC, 128, 3)
    offs = np.repeat(np.arange(8, dtype=np.float32) * 512, 8)[None, :]

    nc = _build_nc()
    in_maps = []
    for c in range(NCORES):
        s = slice(c * BPC, (c + 1) * BPC)
        natcat = np.concatenate([hi[s].reshape(BPC * L, D),
                                 lo[s].reshape(BPC * L, D)], axis=0)
        in_maps.append({
            "hthi": hthi[s],
            "natcat": np.ascontiguousarray(natcat),
            "wqk": wqk, "wvc": wvc, "ws": ws,
            "id64": np.eye(64, dtype=bfnp),
            "id64f": np.eye(64, dtype=np.float32),
            "i1": np.ones((1, 1), bfnp), "i1f": np.ones((1, 1), np.float32),
            "offs": offs,
        })
    import time as _time
    _t0 = _time.time()
    res = run_bass_kernel_spmd(nc, in_maps, core_ids=list(range(NCORES)))
    if os.environ.get("KERNEL_TIME"):
        print(f"[kernel] device dispatch+exec wall: {_time.time() - _t0:.3f}s")
    sl = np.concatenate([r["sl"].astype(np.float32) for r in res.results], 0)
    el = np.concatenate([r["el"].astype(np.float32) for r in res.results], 0)
    return (sl + b_start[0]).astype(np.float32), el.astype(np.float32)


# revision 3
# speedup vs baseline: 1.0689x; 1.0689x over previous
import sys, os
sys.path.insert(0, "/opt/trn_rl_repo")
import numpy as np
import ml_dtypes
from contextlib import ExitStack

import concourse.bass as bass
import concourse.bacc as bacc
import concourse.tile as tile
from concourse import mybir
from concourse.bass_utils import run_bass_kernel_spmd

f32 = mybir.dt.float32
bf16 = mybir.dt.bfloat16
u32 = mybir.dt.uint32
AF = mybir.ActivationFunctionType
ALU = mybir.AluOpType
AX = mybir.AxisListType
bfnp = ml_dtypes.bfloat16

B, L, D, K = 16, 4096, 1024, 5
NCORES = 8
BPC = B // NCORES
LC, DC = L // 128, D // 128
NL8 = L // 512
SCALE = 1.0 / float(np.sqrt(D))

_NC_CACHE = {}


def _build_nc():
    if "nc" in _NC_CACHE:
        return _NC_CACHE["nc"]
    nc = bacc.Bacc("TRN2", target_bir_lowering=False, debug=False,
                   num_devices=NCORES)
    dI = lambda n, s, dt=bf16: nc.dram_tensor(n, s, dt, kind="ExternalInput").ap()
    hthi_d = dI("hthi", [BPC, D, L])            # H^T hi, d-major
    natcat_d = dI("natcat", [2 * BPC * L, D])   # [hi rows; lo rows], natural
    wqk_d = dI("wqk", [D, D])                   # (w_q @ w_k.T) * SCALE
    wvc_d = dI("wvc", [D, D])                   # (w_v @ w_cmp.T) * SCALE
    ws_d = dI("ws", [DC, 128, 3])               # [wsh, wsl, wsh]
    id64_d = dI("id64", [64, 64])
    id64f_d = dI("id64f", [64, 64], f32)
    i1_d = dI("i1", [1, 1])
    i1f_d = dI("i1f", [1, 1], f32)
    offs_d = dI("offs", [1, 64], f32)           # offs[0, c*8+j] = 512*c
    scr_d = nc.dram_tensor("scr", [BPC * 64, 1], f32, kind="Internal").ap()
    sl_d = nc.dram_tensor("sl", [BPC, L], bf16, kind="ExternalOutput").ap()
    el_d = nc.dram_tensor("el", [BPC, L], bf16, kind="ExternalOutput").ap()

    NQ = 4                                       # ht quarter tiles
    QL = L // NQ                                 # 1024 l per quarter

    with tile.TileContext(nc) as tc, ExitStack() as ctx:
        res = ctx.enter_context(tc.tile_pool(name="res", bufs=1))
        stg = ctx.enter_context(tc.tile_pool(name="stg", bufs=4))
        sm = ctx.enter_context(tc.tile_pool(name="sm", bufs=1))
        psc = ctx.enter_context(tc.tile_pool(name="psc", bufs=4, space="PSUM"))
        psm = ctx.enter_context(tc.tile_pool(name="psm", bufs=1, space="PSUM"))
        pss = ctx.enter_context(tc.tile_pool(name="pss", bufs=1, space="PSUM"))

        # ---- resident tiles
        wqk_sb = res.tile([128, DC, D], bf16)
        wvc_sb = res.tile([128, DC, D], bf16)
        ws_sb = res.tile([128, DC, 3], bf16)
        id64 = res.tile([64, 64], bf16)
        id64f = res.tile([64, 64], f32)
        i1 = res.tile([1, 1], bf16)
        i1f = res.tile([1, 1], f32)
        offs = res.tile([1, 64], f32)
        ht_sb = [[res.tile([128, DC, QL], bf16, tag=f"ht{b}q{q}",
                           name=f"ht{b}q{q}") for q in range(NQ)]
                 for b in range(BPC)]

        def htc(b, lc8):
            # [128, 512] rhs slice for l-chunk lc8, per dc
            q, off = lc8 * NQ // NL8, (lc8 % (NL8 // NQ)) * 512
            return lambda dc: ht_sb[b][q][:, dc, off:off + 512]

        def load_ht(b):
            for q in range(NQ):
                for dc in range(DC):
                    nc.sync.dma_start(
                        ht_sb[b][q][:, dc, :],
                        hthi_d[b, dc * 128:(dc + 1) * 128,
                               q * QL:(q + 1) * QL])

        def load_small_consts():
            for dc in range(DC):
                nc.sync.dma_start(ws_sb[:, dc, :], ws_d[dc])
            nc.sync.dma_start(id64[:], id64_d[:])
            nc.sync.dma_start(id64f[:], id64f_d[:])
            nc.sync.dma_start(i1[:], i1_d[:])
            nc.sync.dma_start(i1f[:], i1f_d[:])
            nc.sync.dma_start(offs[:], offs_d[:])

        def load_weights():
            for dc in range(DC):
                nc.scalar.dma_start(wqk_sb[:, dc, :],
                                    wqk_d[dc * 128:(dc + 1) * 128, :])
            for dc in range(DC):
                nc.scalar.dma_start(wvc_sb[:, dc, :],
                                    wvc_d[dc * 128:(dc + 1) * 128, :])

        st = [dict() for _ in range(BPC)]   # per-example cross-phase tiles

        def s1_start_logits(b):
            ci64 = sm.tile([1, 64], f32, tag=f"ci64_{b}", name=f"ci64_{b}")
            st[b]["ci64"] = ci64

            for lc8 in range(NL8):
                rhs = htc(b, lc8)
                pc = psc.tile([64, 512], f32, tag="chunk", name="pc")
                for dc in range(DC):
                    _mm = nc.tensor.matmul(pc[0:1, :], ws_sb[:, dc, 0:1],
                                           rhs(dc), start=(dc == 0),
                                           stop=(dc == DC - 1))
                    if b == 0 and lc8 == 0 and os.environ.get("TILE_FOLLOW"):
                        tile.tile_follow(_mm, log_all_deps=True)
                sls1 = sm.tile([1, 512], bf16, tag="els", name="sls1", bufs=2)
                nc.scalar.copy(sls1[:], pc[0:1, :])
                nc.scalar.dma_start(sl_d[b:b + 1, lc8 * 512:(lc8 + 1) * 512],
                                    sls1[:])
                c8v = sm.tile([1, 8], f32, tag="c8v", name="c8v")
                c8i = sm.tile([1, 8], u32, tag="c8i", name="c8i")
                nc.vector.max(c8v[:], pc[0:1, :])
                nc.vector.max_index(c8i[:], c8v[:], pc[0:1, :])
                nc.vector.tensor_copy(ci64[0:1, lc8 * 8:(lc8 + 1) * 8], c8i[:])

        def s2a_gather(b):
            # non-PE: candidate l list -> [64,1] index tiles -> row gathers
            ci64 = st[b]["ci64"]
            nc.vector.tensor_tensor(ci64[:], ci64[:], offs[:], ALU.add)
            cif_t = sm.tile([64, 1], f32, tag="cif_t", name="cif_t")
            nc.scalar.dma_start(
                scr_d[b * 64:(b + 1) * 64].rearrange("a x -> x a"),
                ci64[0:1, :])
            nc.scalar.dma_start(cif_t[:], scr_d[b * 64:(b + 1) * 64])
            candl = sm.tile([64, 1], u32, tag="candl", name="candl")
            nc.vector.tensor_scalar(candl[:], cif_t[:], float(b * L), None,
                                    ALU.add)
            candlb = sm.tile([64, 1], u32, tag="candlb", name="candlb")
            nc.vector.tensor_scalar(candlb[:], cif_t[:],
                                    float(b * L + BPC * L), None, ALU.add)
            hi64 = sm.tile([64, D], bf16, tag="hi64", name="hi64")
            lo64 = sm.tile([64, D], bf16, tag="lo64", name="lo64")
            st[b]["hi64"], st[b]["lo64"] = hi64, lo64
            nc.gpsimd.indirect_dma_start(
                out=hi64[:], out_offset=None, in_=natcat_d[:],
                in_offset=bass.IndirectOffsetOnAxis(ap=candl[:, 0:1], axis=0))
            nc.gpsimd.indirect_dma_start(
                out=lo64[:], out_offset=None, in_=natcat_d[:],
                in_offset=bass.IndirectOffsetOnAxis(ap=candlb[:, 0:1], axis=0))

        def s2b_refine(b):
            hi64, lo64 = st[b]["hi64"], st[b]["lo64"]
            phi = pss.tile([128, DC, 64], bf16, tag="smb", name="phi")
            for dc in range(DC):
                nc.tensor.transpose(phi[:, dc, :],
                                    hi64[:, dc * 128:(dc + 1) * 128], id64[:])
            hi64T = sm.tile([128, DC, 64], bf16, tag="hi64T", name="hi64T")
            st[b]["hi64T"] = hi64T
            nc.vector.tensor_copy(hi64T[:], phi[:])
            plo = pss.tile([128, DC, 64], bf16, tag="smb", name="plo")
            for dc in range(DC):
                nc.tensor.transpose(plo[:, dc, :],
                                    lo64[:, dc * 128:(dc + 1) * 128], id64[:])
            lo64T = sm.tile([128, DC, 64], bf16, tag="lo64T", name="lo64T")
            nc.scalar.copy(lo64T[:], plo[:])

            pcor = pss.tile([128, 64], f32, tag="smf", name="pcor")
            for dc in range(DC):
                nc.tensor.matmul(pcor[0:64, 0:2], hi64T[:, dc, :],
                                 ws_sb[:, dc, 0:2],
                                 start=(dc == 0), stop=(dc == DC - 1),
                                 skip_group_check=True)
                nc.tensor.matmul(pcor[0:64, 2:3], lo64T[:, dc, :],
                                 ws_sb[:, dc, 2:3],
                                 start=False, stop=(dc == DC - 1),
                                 skip_group_check=True)
            ex64p = sm.tile([64, 1], f32, tag=f"ex64p_{b}", name=f"ex64p_{b}")
            st[b]["ex64p"] = ex64p
            nc.vector.tensor_reduce(ex64p[:], pcor[0:64, 0:3], AX.X, ALU.add)

        def s2c_weights(b):
            ex64p = st[b]["ex64p"]
            pex = pss.tile([128, 64], f32, tag="smf", name="pex")
            nc.tensor.transpose(pex[0:1, 0:64], ex64p[:], id64f[:])
            ex64f = sm.tile([1, 64], f32, tag="ex64f", name="ex64f")
            nc.vector.tensor_copy(ex64f[:], pex[0:1, 0:64])
            top8v = sm.tile([1, 8], f32, tag="top8v", name="top8v")
            nc.vector.max(top8v[:], ex64f[:])
            mask64 = sm.tile([1, 64], f32, tag="mask64", name="mask64")
            nc.vector.tensor_scalar(mask64[:], ex64f[:], top8v[0:1, 4:5],
                                    None, ALU.is_ge)
            e64 = sm.tile([1, 64], f32, tag="e64", name="e64")
            nc.scalar.activation(e64[:], ex64f[:], AF.Exp)
            nc.vector.tensor_tensor(e64[:], e64[:], mask64[:], ALU.mult)
            s64 = sm.tile([1, 1], f32, tag="s64", name="s64")
            nc.vector.tensor_reduce(s64[:], e64[:], AX.X, ALU.add)
            rs64 = sm.tile([1, 1], f32, tag="rs64", name="rs64")
            nc.vector.reciprocal(rs64[:], s64[:])
            w64 = sm.tile([1, 64], f32, tag="w64", name="w64")
            nc.vector.tensor_scalar_mul(w64[:], e64[:], rs64[:])
            pw = pss.tile([128, 64], f32, tag="smf", name="pw")
            nc.tensor.transpose(pw[0:64, 0:1], w64[:], i1f[:])
            w64T = sm.tile([64, 1], f32, tag=f"w64T_{b}", name=f"w64T_{b}")
            st[b]["w64T"] = w64T
            nc.vector.tensor_copy(w64T[:], pw[0:64, 0:1])

        def s4_P(b):
            hi64T = st[b]["hi64T"]
            pP = psm.tile([64, D], f32, tag="mid", name="pP")
            for o in range(2):
                for dci in range(DC):
                    nc.tensor.matmul(pP[:, o * 512:(o + 1) * 512],
                                     hi64T[:, dci, :],
                                     wqk_sb[:, dci, o * 512:(o + 1) * 512],
                                     start=(dci == 0), stop=(dci == DC - 1),
                                     skip_group_check=True)
            P_sb = sm.tile([64, D], bf16, tag="P_sb", name="P_sb")
            nc.scalar.copy(P_sb[:, 0:512], pP[:, 0:512])
            nc.scalar.copy(P_sb[:, 512:D], pP[:, 512:D])
            pPT = pss.tile([128, DC, 64], bf16, tag="smb", name="pPT")
            for dc in range(DC):
                nc.tensor.transpose(pPT[:, dc, :],
                                    P_sb[:, dc * 128:(dc + 1) * 128], id64[:])
            PT = sm.tile([128, DC, 64], bf16, tag=f"PT_{b}", name=f"PT_{b}")
            st[b]["PT"] = PT
            nc.vector.tensor_copy(PT[:], pPT[:])

        def s5_scores(b):
            PT, w64T = st[b]["PT"], st[b]["w64T"]
            E_sb = sm.tile([64, L], bf16, tag="E_sb", name="E_sb")
            z64a = sm.tile([64, NL8], f32, tag="z64a", name="z64a")
            for lc8 in range(NL8):
                rhs = htc(b, lc8)
                psc5 = psc.tile([64, 512], f32, tag="chunk", name="psc5")
                for dc in range(DC):
                    nc.tensor.matmul(psc5[:], PT[:, dc, :], rhs(dc),
                                     start=(dc == 0), stop=(dc == DC - 1))
                nc.scalar.activation(E_sb[:, lc8 * 512:(lc8 + 1) * 512],
                                     psc5[:], AF.Exp,
                                     accum_out=z64a[:, lc8:lc8 + 1])
            z64 = sm.tile([64, 1], f32, tag="z64", name="z64")
            nc.vector.tensor_reduce(z64[:], z64a[:], AX.X, ALU.add)
            rz64 = sm.tile([64, 1], f32, tag="rz64", name="rz64")
            nc.vector.reciprocal(rz64[:], z64[:])
            c64 = sm.tile([64, 1], bf16, tag="c64", name="c64")
            nc.vector.tensor_tensor(c64[:], w64T[:], rz64[:], ALU.mult)
            pm32 = pss.tile([128, 64], f32, tag="smf", name="pm32")
            for c in range(LC):
                nc.tensor.matmul(pm32[:, c:c + 1],
                                 E_sb[:, c * 128:(c + 1) * 128], c64[:],
                                 start=True, stop=True, skip_group_check=True)
            m32 = sm.tile([128, LC], bf16, tag=f"m32_{b}", name=f"m32_{b}")
            st[b]["m32"] = m32
            nc.vector.tensor_copy(m32[:], pm32[:, 0:LC])

        def s6_amix(b):
            m32 = st[b]["m32"]
            pam = psm.tile([64, D], f32, tag="mid", name="pam")
            for lcp in range(LC // 2):
                natc = stg.tile([128, 2, D], bf16, tag="natstg", name="natc")
                nc.sync.dma_start(
                    natc[:],
                    natcat_d[b * L + lcp * 256:b * L + (lcp + 1) * 256, :]
                    .rearrange("(t p) d -> p t d", p=128))
                for t in range(2):
                    lc = lcp * 2 + t
                    for o in range(2):
                        nc.tensor.matmul(pam[0:1, o * 512:(o + 1) * 512],
                                         m32[:, lc:lc + 1],
                                         natc[:, t, o * 512:(o + 1) * 512],
                                         start=(lc == 0), stop=(lc == LC - 1),
                                         skip_group_check=True)
            amix = sm.tile([1, D], bf16, tag="amix", name="amix")
            nc.scalar.copy(amix[0:1, 0:512], pam[0:1, 0:512])
            nc.scalar.copy(amix[0:1, 512:D], pam[0:1, 512:D])
            pamT = pss.tile([128, DC, 64], bf16, tag="smb", name="pamT")
            for dc in range(DC):
                nc.tensor.transpose(pamT[:, dc, 0:1],
                                    amix[0:1, dc * 128:(dc + 1) * 128], i1[:])
            amixT = sm.tile([128, DC, 1], bf16, tag=f"amixT_{b}",
                            name=f"amixT_{b}")
            st[b]["amixT"] = amixT
            nc.vector.tensor_copy(amixT[:], pamT[:, :, 0:1])

        def s7_g(b):
            amixT = st[b]["amixT"]
            pg = psm.tile([64, D], f32, tag="mid", name="pg")
            for o in range(2):
                for dci in range(DC):
                    nc.tensor.matmul(pg[0:1, o * 512:(o + 1) * 512],
                                     amixT[:, dci, :],
                                     wvc_sb[:, dci, o * 512:(o + 1) * 512],
                                     start=(dci == 0), stop=(dci == DC - 1),
                                     skip_group_check=True)
            g_sb = sm.tile([1, D], bf16, tag="g_sb", name="g_sb")
            nc.scalar.copy(g_sb[0:1, 0:512], pg[0:1, 0:512])
            nc.scalar.copy(g_sb[0:1, 512:D], pg[0:1, 512:D])
            pgT = pss.tile([128, DC, 64], bf16, tag="smb", name="pgT")
            for dc in range(DC):
                nc.tensor.transpose(pgT[:, dc, 0:1],
                                    g_sb[0:1, dc * 128:(dc + 1) * 128], i1[:])
            gT = sm.tile([128, DC, 1], bf16, tag=f"gT_{b}", name=f"gT_{b}")
            st[b]["gT"] = gT
            nc.vector.tensor_copy(gT[:], pgT[:, :, 0:1])

        def s8_el(b):
            gT = st[b]["gT"]
            for lc8 in range(NL8):
                rhs = htc(b, lc8)
                pe = psc.tile([64, 512], f32, tag="chunk", name="pe")
                for dc in range(DC):
                    nc.tensor.matmul(pe[0:1, :], gT[:, dc, :], rhs(dc),
                                     start=(dc == 0), stop=(dc == DC - 1))
                els = sm.tile([1, 512], bf16, tag="els", name="els", bufs=2)
                nc.vector.tensor_copy(els[:], pe[0:1, :])
                nc.scalar.dma_start(el_d[b:b + 1, lc8 * 512:(lc8 + 1) * 512],
                                    els[:])

        # ---------------- schedule ----------------
        load_small_consts()
        load_ht(0)
        load_ht(1)
        load_weights()
        s1_start_logits(0)
        s2a_gather(0)
        s1_start_logits(1)
        s2b_refine(0)
        s2a_gather(1)
        s4_P(0)
        s2c_weights(0)
        s2b_refine(1)
        s4_P(1)
        s2c_weights(1)
        s5_scores(0)
        s5_scores(1)
        s6_amix(0)
        s7_g(0)
        s6_amix(1)
        s8_el(0)
        s7_g(1)
        s8_el(1)

    nc.compile()
    _NC_CACHE["nc"] = nc
    return nc


def _np_reference(H, attention_mask, w_start, b_start, w_q, b_q, w_k, b_k,
                  w_v, b_v, w_cmp, b_cmp):
    NEG = -1e9
    H = H.astype(np.float32)
    pad = attention_mask == 0
    sl = (H @ w_start + b_start)[..., 0]
    sl = np.where(pad, NEG, sl)
    x = sl - sl.max(-1, keepdims=True)
    e = np.exp(x); sp = e / e.sum(-1, keepdims=True)
    idx = np.argsort(-sp, axis=-1, kind="stable")[:, :K]
    tp = np.take_along_axis(sp, idx, axis=1)
    sr = np.take_along_axis(H, idx[..., None], axis=1)
    Q = sr @ w_q + b_q
    K_ = H @ w_k + b_k
    V = H @ w_v + b_v
    sc = np.einsum('bkd,bld->bkl', Q, K_) * SCALE
    sc = np.where(pad[:, None, :], NEG, sc)
    sc = sc - sc.max(-1, keepdims=True)
    a = np.exp(sc); a = a / a.sum(-1, keepdims=True)
    ctx_ = np.einsum('bkl,bld->bkd', a, V)
    tcmp = H @ w_cmp + b_cmp
    es = np.einsum('bkd,bld->bkl', ctx_, tcmp) * SCALE
    es = np.where(pad[:, None, :], NEG, es)
    w = tp / (tp.sum(-1, keepdims=True) + 1e-9)
    el = np.einsum('bk,bkl->bl', w, es)
    el = np.where(pad, NEG, el)
    return sl, el


def kernel(**inputs):
    H = np.asarray(inputs["H"], np.float32)
    mask = np.asarray(inputs["attention_mask"])
    b_start = np.asarray(inputs["b_start"], np.float32)
    biases_zero = all(np.all(np.asarray(inputs[n]) == 0)
                      for n in ["b_q", "b_k", "b_v", "b_cmp"])
    if not bool((mask == 1).all()) or not biases_zero:
        sl, el = _np_reference(**{k: np.asarray(v) for k, v in inputs.items()})
        return np.asarray(sl, np.float32), np.asarray(el, np.float32)

    w_start = np.asarray(inputs["w_start"], np.float32)
    w_q = np.asarray(inputs["w_q"], np.float32)
    w_k = np.asarray(inputs["w_k"], np.float32)
    w_v = np.asarray(inputs["w_v"], np.float32)
    w_cmp = np.asarray(inputs["w_cmp"], np.float32)

    hi = H.astype(bfnp)
    lo = (H - hi.astype(np.float32)).astype(bfnp)
    hthi = np.ascontiguousarray(hi.transpose(0, 2, 1))            # [B, D, L]
    wqk = ((w_q @ w_k.T) * SCALE).astype(bfnp)
    wvc = ((w_v @ w_cmp.T) * SCALE).astype(bfnp)
    wsh = w_start[:, 0].astype(bfnp)
    wsl = (w_start[:, 0] - wsh.astype(np.float32)).astype(bfnp)
    ws = np.stack([wsh, wsl, wsh], axis=-1).reshape(DC, 128, 3)
    offs = np.repeat(np.arange(8, dtype=np.float32) * 512, 8)[None, :]

    nc = _build_nc()
    in_maps = []
    for c in range(NCORES):
        s = slice(c * BPC, (c + 1) * BPC)
        natcat = np.concatenate([hi[s].reshape(BPC * L, D),
                                 lo[s].reshape(BPC * L, D)], axis=0)
        in_maps.append({
            "hthi": hthi[s],
            "natcat": np.ascontiguousarray(natcat),
            "wqk": wqk, "wvc": wvc, "ws": ws,
            "id64": np.eye(64, dtype=bfnp),
            "id64f": np.eye(64, dtype=np.float32),
            "i1": np.ones((1, 1), bfnp), "i1f": np.ones((1, 1), np.float32),
            "offs": offs,
        })
    import time as _time
    _t0 = _time.time()
    res = run_bass_kernel_spmd(nc, in_maps, core_ids=list(range(NCORES)))
    if os.environ.get("KERNEL_TIME"):
        print(f"[kernel] device dispatch+exec wall: {_time.time() - _t0:.3f}s")
    sl = np.concatenate([r["sl"].astype(np.float32) for r in res.results], 0)
    el = np.concatenate([r["el"].astype(np.float32) for r in res.results], 0)
    return (sl + b_start[0]).astype(np.float32), el.astype(np.float32)


# revision 4
# speedup vs baseline: 1.0788x; 1.0093x over previous
import sys, os
sys.path.insert(0, "/opt/trn_rl_repo")
import numpy as np
import ml_dtypes
from contextlib import ExitStack

import concourse.bass as bass
import concourse.bacc as bacc
import concourse.tile as tile
from concourse import mybir
from concourse.bass_utils import run_bass_kernel_spmd

f32 = mybir.dt.float32
bf16 = mybir.dt.bfloat16
u32 = mybir.dt.uint32
AF = mybir.ActivationFunctionType
ALU = mybir.AluOpType
AX = mybir.AxisListType
bfnp = ml_dtypes.bfloat16

B, L, D, K = 16, 4096, 1024, 5
NCORES = 8
BPC = B // NCORES
LC, DC = L // 128, D // 128
NL8 = L // 512
SCALE = 1.0 / float(np.sqrt(D))

_NC_CACHE = {}


def _build_nc():
    if "nc" in _NC_CACHE:
        return _NC_CACHE["nc"]
    nc = bacc.Bacc("TRN2", target_bir_lowering=False, debug=False,
                   num_devices=NCORES)
    dI = lambda n, s, dt=bf16: nc.dram_tensor(n, s, dt, kind="ExternalInput").ap()
    hthi_d = dI("hthi", [BPC, D, L])            # H^T hi, d-major
    natcat_d = dI("natcat", [2 * BPC * L, D])   # [hi rows; lo rows], natural
    wqk_d = dI("wqk", [D, D])                   # (w_q @ w_k.T) * SCALE
    wvc_d = dI("wvc", [D, D])                   # (w_v @ w_cmp.T) * SCALE
    ws_d = dI("ws", [DC, 128, 3])               # [wsh, wsl, wsh]
    id64_d = dI("id64", [64, 64])
    id64f_d = dI("id64f", [64, 64], f32)
    i1_d = dI("i1", [1, 1])
    i1f_d = dI("i1f", [1, 1], f32)
    offs_d = dI("offs", [1, 64], f32)           # offs[0, c*8+j] = 512*c
    scr_d = nc.dram_tensor("scr", [BPC * 64, 1], f32, kind="Internal").ap()
    sl_d = nc.dram_tensor("sl", [BPC, L], bf16, kind="ExternalOutput").ap()
    el_d = nc.dram_tensor("el", [BPC, L], bf16, kind="ExternalOutput").ap()

    NQ = 4                                       # ht quarter tiles
    QL = L // NQ                                 # 1024 l per quarter

    with tile.TileContext(nc) as tc, ExitStack() as ctx:
        res = ctx.enter_context(tc.tile_pool(name="res", bufs=1))
        stg = ctx.enter_context(tc.tile_pool(name="stg", bufs=4))
        sm = ctx.enter_context(tc.tile_pool(name="sm", bufs=1))
        psc = ctx.enter_context(tc.tile_pool(name="psc", bufs=4, space="PSUM"))
        psm = ctx.enter_context(tc.tile_pool(name="psm", bufs=1, space="PSUM"))
        pss = ctx.enter_context(tc.tile_pool(name="pss", bufs=1, space="PSUM"))

        # ---- resident tiles
        wqk_sb = res.tile([128, DC, D], bf16)
        wvc_sb = res.tile([128, DC, D], bf16)
        ws_sb = res.tile([128, DC, 3], bf16)
        id64 = res.tile([64, 64], bf16)
        id64f = res.tile([64, 64], f32)
        i1 = res.tile([1, 1], bf16)
        i1f = res.tile([1, 1], f32)
        offs = res.tile([1, 64], f32)
        ht_sb = [[res.tile([128, DC, QL], bf16, tag=f"ht{b}q{q}",
                           name=f"ht{b}q{q}") for q in range(NQ)]
                 for b in range(BPC)]

        def htc(b, lc8):
            # [128, 512] rhs slice for l-chunk lc8, per dc
            q, off = lc8 * NQ // NL8, (lc8 % (NL8 // NQ)) * 512
            return lambda dc: ht_sb[b][q][:, dc, off:off + 512]

        def load_ht(b, qs=None):
            for q in (range(NQ) if qs is None else qs):
                for dc in range(DC):
                    nc.sync.dma_start(
                        ht_sb[b][q][:, dc, :],
                        hthi_d[b, dc * 128:(dc + 1) * 128,
                               q * QL:(q + 1) * QL])

        def load_small_consts():
            for dc in range(DC):
                nc.sync.dma_start(ws_sb[:, dc, :], ws_d[dc])
            nc.sync.dma_start(id64[:], id64_d[:])
            nc.sync.dma_start(id64f[:], id64f_d[:])
            nc.sync.dma_start(i1[:], i1_d[:])
            nc.sync.dma_start(i1f[:], i1f_d[:])
            nc.sync.dma_start(offs[:], offs_d[:])

        def load_weights():
            for dc in range(DC):
                nc.scalar.dma_start(wqk_sb[:, dc, :],
                                    wqk_d[dc * 128:(dc + 1) * 128, :])
            for dc in range(DC):
                nc.scalar.dma_start(wvc_sb[:, dc, :],
                                    wvc_d[dc * 128:(dc + 1) * 128, :])

        st = [dict() for _ in range(BPC)]   # per-example cross-phase tiles

        def s1_start_logits(b, chunks=None):
            if chunks is None:
                chunks = range(NL8)
            if "ci64" not in st[b]:
                st[b]["ci64"] = sm.tile([1, 64], f32, tag=f"ci64_{b}",
                                        name=f"ci64_{b}")
            ci64 = st[b]["ci64"]

            for lc8 in chunks:
                rhs = htc(b, lc8)
                pc = psc.tile([64, 512], f32, tag="chunk", name="pc")
                for dc in range(DC):
                    _mm = nc.tensor.matmul(pc[0:1, :], ws_sb[:, dc, 0:1],
                                           rhs(dc), start=(dc == 0),
                                           stop=(dc == DC - 1))
                    if b == 0 and lc8 == 0 and os.environ.get("TILE_FOLLOW"):
                        tile.tile_follow(_mm, log_all_deps=True)
                sls1 = sm.tile([1, 512], bf16, tag="sls1", name="sls1", bufs=2)
                nc.scalar.copy(sls1[:], pc[0:1, :])
                nc.scalar.dma_start(sl_d[b:b + 1, lc8 * 512:(lc8 + 1) * 512],
                                    sls1[:])
                c8v = sm.tile([1, 8], bf16, tag="c8v", name="c8v")
                c8i = sm.tile([1, 8], u32, tag="c8i", name="c8i")
                nc.vector.max(c8v[:], sls1[:])
                nc.vector.max_index(c8i[:], c8v[:], sls1[:])
                nc.vector.tensor_copy(ci64[0:1, lc8 * 8:(lc8 + 1) * 8], c8i[:])

        def s2a_gather(b):
            # non-PE: candidate l list -> [64,1] index tiles -> row gathers
            ci64 = st[b]["ci64"]
            nc.vector.tensor_tensor(ci64[:], ci64[:], offs[:], ALU.add)
            cif_t = sm.tile([64, 1], f32, tag="cif_t", name="cif_t")
            nc.scalar.dma_start(
                scr_d[b * 64:(b + 1) * 64].rearrange("a x -> x a"),
                ci64[0:1, :])
            nc.scalar.dma_start(cif_t[:], scr_d[b * 64:(b + 1) * 64])
            candl = sm.tile([64, 1], u32, tag="candl", name="candl")
            nc.vector.tensor_scalar(candl[:], cif_t[:], float(b * L), None,
                                    ALU.add)
            candlb = sm.tile([64, 1], u32, tag="candlb", name="candlb")
            nc.vector.tensor_scalar(candlb[:], cif_t[:],
                                    float(b * L + BPC * L), None, ALU.add)
            hi64 = sm.tile([64, D], bf16, tag="hi64", name="hi64")
            lo64 = sm.tile([64, D], bf16, tag="lo64", name="lo64")
            st[b]["hi64"], st[b]["lo64"] = hi64, lo64
            nc.gpsimd.indirect_dma_start(
                out=hi64[:], out_offset=None, in_=natcat_d[:],
                in_offset=bass.IndirectOffsetOnAxis(ap=candl[:, 0:1], axis=0))
            nc.gpsimd.indirect_dma_start(
                out=lo64[:], out_offset=None, in_=natcat_d[:],
                in_offset=bass.IndirectOffsetOnAxis(ap=candlb[:, 0:1], axis=0))

        def s2b_refine(b):
            hi64, lo64 = st[b]["hi64"], st[b]["lo64"]
            phi = pss.tile([128, DC, 64], bf16, tag="smb", name="phi")
            for dc in range(DC):
                nc.tensor.transpose(phi[:, dc, :],
                                    hi64[:, dc * 128:(dc + 1) * 128], id64[:])
            hi64T = sm.tile([128, DC, 64], bf16, tag="hi64T", name="hi64T")
            st[b]["hi64T"] = hi64T
            nc.vector.tensor_copy(hi64T[:, 0:4, :], phi[:, 0:4, :])
            nc.vector.tensor_copy(hi64T[:, 4:DC, :], phi[:, 4:DC, :])
            plo = pss.tile([128, DC, 64], bf16, tag="smb", name="plo")
            for dc in range(DC):
                nc.tensor.transpose(plo[:, dc, :],
                                    lo64[:, dc * 128:(dc + 1) * 128], id64[:])
            lo64T = sm.tile([128, DC, 64], bf16, tag="lo64T", name="lo64T")
            nc.scalar.copy(lo64T[:, 0:4, :], plo[:, 0:4, :])
            nc.scalar.copy(lo64T[:, 4:DC, :], plo[:, 4:DC, :])

            pcor = pss.tile([128, 64], f32, tag="smf", name="pcor")
            for dc in range(DC):
                nc.tensor.matmul(pcor[0:64, 0:2], hi64T[:, dc, :],
                                 ws_sb[:, dc, 0:2],
                                 start=(dc == 0), stop=(dc == DC - 1),
                                 skip_group_check=True)
                nc.tensor.matmul(pcor[0:64, 2:3], lo64T[:, dc, :],
                                 ws_sb[:, dc, 2:3],
                                 start=False, stop=(dc == DC - 1),
                                 skip_group_check=True)
            ex64p = sm.tile([64, 1], f32, tag=f"ex64p_{b}", name=f"ex64p_{b}")
            st[b]["ex64p"] = ex64p
            nc.vector.tensor_reduce(ex64p[:], pcor[0:64, 0:3], AX.X, ALU.add)

        def s2c_weights(b):
            ex64p = st[b]["ex64p"]
            pex = pss.tile([128, 64], f32, tag="smf", name="pex")
            nc.tensor.transpose(pex[0:1, 0:64], ex64p[:], id64f[:])
            ex64f = sm.tile([1, 64], f32, tag="ex64f", name="ex64f")
            nc.vector.tensor_copy(ex64f[:], pex[0:1, 0:64])
            top8v = sm.tile([1, 8], f32, tag="top8v", name="top8v")
            nc.vector.max(top8v[:], ex64f[:])
            mask64 = sm.tile([1, 64], f32, tag="mask64", name="mask64")
            nc.vector.tensor_scalar(mask64[:], ex64f[:], top8v[0:1, 4:5],
                                    None, ALU.is_ge)
            e64 = sm.tile([1, 64], f32, tag="e64", name="e64")
            nc.scalar.activation(e64[:], ex64f[:], AF.Exp)
            nc.vector.tensor_tensor(e64[:], e64[:], mask64[:], ALU.mult)
            s64 = sm.tile([1, 1], f32, tag="s64", name="s64")
            nc.vector.tensor_reduce(s64[:], e64[:], AX.X, ALU.add)
            rs64 = sm.tile([1, 1], f32, tag="rs64", name="rs64")
            nc.vector.reciprocal(rs64[:], s64[:])
            w64 = sm.tile([1, 64], f32, tag="w64", name="w64")
            nc.vector.tensor_scalar_mul(w64[:], e64[:], rs64[:])
            pw = pss.tile([128, 64], f32, tag="smf", name="pw")
            nc.tensor.transpose(pw[0:64, 0:1], w64[:], i1f[:])
            w64T = sm.tile([64, 1], f32, tag=f"w64T_{b}", name=f"w64T_{b}")
            st[b]["w64T"] = w64T
            nc.vector.tensor_copy(w64T[:], pw[0:64, 0:1])

        def s4_P(b):
            hi64T = st[b]["hi64T"]
            pP = psm.tile([64, D], f32, tag="mid", name="pP")
            for o in range(2):
                for dci in range(DC):
                    nc.tensor.matmul(pP[:, o * 512:(o + 1) * 512],
                                     hi64T[:, dci, :],
                                     wqk_sb[:, dci, o * 512:(o + 1) * 512],
                                     start=(dci == 0), stop=(dci == DC - 1),
                                     skip_group_check=True)
            P_sb = sm.tile([64, D], bf16, tag="P_sb", name="P_sb")
            nc.scalar.copy(P_sb[:, 0:512], pP[:, 0:512])
            nc.scalar.copy(P_sb[:, 512:D], pP[:, 512:D])
            pPT = pss.tile([128, DC, 64], bf16, tag="smb", name="pPT")
            for dc in range(DC):
                nc.tensor.transpose(pPT[:, dc, :],
                                    P_sb[:, dc * 128:(dc + 1) * 128], id64[:])
            PT = sm.tile([128, DC, 64], bf16, tag=f"PT_{b}", name=f"PT_{b}")
            st[b]["PT"] = PT
            nc.vector.tensor_copy(PT[:], pPT[:])

        def s5_scores(b):
            PT, w64T = st[b]["PT"], st[b]["w64T"]
            E_sb = sm.tile([64, L], bf16, tag="E_sb", name="E_sb")
            z64a = sm.tile([64, NL8], f32, tag="z64a", name="z64a")
            for lc8 in range(NL8):
                rhs = htc(b, lc8)
                psc5 = psc.tile([64, 512], f32, tag="chunk", name="psc5")
                for dc in range(DC):
                    nc.tensor.matmul(psc5[:], PT[:, dc, :], rhs(dc),
                                     start=(dc == 0), stop=(dc == DC - 1))
                nc.scalar.activation(E_sb[:, lc8 * 512:(lc8 + 1) * 512],
                                     psc5[:], AF.Exp,
                                     accum_out=z64a[:, lc8:lc8 + 1])
            z64 = sm.tile([64, 1], f32, tag="z64", name="z64")
            nc.vector.tensor_reduce(z64[:], z64a[:], AX.X, ALU.add)
            rz64 = sm.tile([64, 1], f32, tag="rz64", name="rz64")
            nc.vector.reciprocal(rz64[:], z64[:])
            c64 = sm.tile([64, 1], bf16, tag="c64", name="c64")
            nc.vector.tensor_tensor(c64[:], w64T[:], rz64[:], ALU.mult)
            pm32 = pss.tile([128, 64], f32, tag="smf", name="pm32")
            for c in range(LC):
                nc.tensor.matmul(pm32[:, c:c + 1],
                                 E_sb[:, c * 128:(c + 1) * 128], c64[:],
                                 start=True, stop=True, skip_group_check=True)
            m32 = sm.tile([128, LC], bf16, tag=f"m32_{b}", name=f"m32_{b}")
            st[b]["m32"] = m32
            nc.vector.tensor_copy(m32[:], pm32[:, 0:LC])

        def s6_amix(b):
            m32 = st[b]["m32"]
            pam = psm.tile([64, D], f32, tag="mid", name="pam")
            for lcp in range(LC // 2):
                natc = stg.tile([128, 2, D], bf16, tag="natstg", name="natc")
                nc.sync.dma_start(
                    natc[:],
                    natcat_d[b * L + lcp * 256:b * L + (lcp + 1) * 256, :]
                    .rearrange("(t p) d -> p t d", p=128))
                for t in range(2):
                    lc = lcp * 2 + t
                    for o in range(2):
                        nc.tensor.matmul(pam[0:1, o * 512:(o + 1) * 512],
                                         m32[:, lc:lc + 1],
                                         natc[:, t, o * 512:(o + 1) * 512],
                                         start=(lc == 0), stop=(lc == LC - 1),
                                         skip_group_check=True)
            amix = sm.tile([1, D], bf16, tag="amix", name="amix")
            nc.scalar.copy(amix[0:1, 0:512], pam[0:1, 0:512])
            nc.scalar.copy(amix[0:1, 512:D], pam[0:1, 512:D])
            pamT = pss.tile([128, DC, 64], bf16, tag="smb", name="pamT")
            for dc in range(DC):
                nc.tensor.transpose(pamT[:, dc, 0:1],
                                    amix[0:1, dc * 128:(dc + 1) * 128], i1[:])
            amixT = sm.tile([128, DC, 1], bf16, tag=f"amixT_{b}",
                            name=f"amixT_{b}")
            st[b]["amixT"] = amixT
            nc.vector.tensor_copy(amixT[:], pamT[:, :, 0:1])

        def s7_g(b):
            amixT = st[b]["amixT"]
            pg = psm.tile([64, D], f32, tag="mid", name="pg")
            for o in range(2):
                for dci in range(DC):
                    nc.tensor.matmul(pg[0:1, o * 512:(o + 1) * 512],
                                     amixT[:, dci, :],
                                     wvc_sb[:, dci, o * 512:(o + 1) * 512],
                                     start=(dci == 0), stop=(dci == DC - 1),
                                     skip_group_check=True)
            g_sb = sm.tile([1, D], bf16, tag="g_sb", name="g_sb")
            nc.scalar.copy(g_sb[0:1, 0:512], pg[0:1, 0:512])
            nc.scalar.copy(g_sb[0:1, 512:D], pg[0:1, 512:D])
            pgT = pss.tile([128, DC, 64], bf16, tag="smb", name="pgT")
            for dc in range(DC):
                nc.tensor.transpose(pgT[:, dc, 0:1],
                                    g_sb[0:1, dc * 128:(dc + 1) * 128], i1[:])
            gT = sm.tile([128, DC, 1], bf16, tag=f"gT_{b}", name=f"gT_{b}")
            st[b]["gT"] = gT
            nc.vector.tensor_copy(gT[:], pgT[:, :, 0:1])

        def s8_el(b):
            gT = st[b]["gT"]
            for lc8 in range(NL8):
                rhs = htc(b, lc8)
                pe = psc.tile([64, 512], f32, tag="chunk", name="pe")
                for dc in range(DC):
                    nc.tensor.matmul(pe[0:1, :], gT[:, dc, :], rhs(dc),
                                     start=(dc == 0), stop=(dc == DC - 1))
                els = sm.tile([1, 512], bf16, tag="els", name="els", bufs=2)
                nc.vector.tensor_copy(els[:], pe[0:1, :])
                nc.scalar.dma_start(el_d[b:b + 1, lc8 * 512:(lc8 + 1) * 512],
                                    els[:])

        # ---------------- schedule ----------------
        load_small_consts()
        load_ht(0, [0, 1])
        s1_start_logits(0, [0, 1])
        load_ht(0, [2])
        s1_start_logits(0, [2, 3])
        load_ht(0, [3])
        s1_start_logits(0, [4, 5])
        load_ht(1, [0])
        s1_start_logits(0, [6, 7])
        s2a_gather(0)
        load_ht(1, [1, 2])
        s1_start_logits(1, [0, 1])
        load_ht(1, [3])
        load_weights()
        s1_start_logits(1, [2, 3, 4, 5, 6, 7])
        s2b_refine(0)
        s2a_gather(1)
        s4_P(0)
        s2c_weights(0)
        s2b_refine(1)
        s4_P(1)
        s2c_weights(1)
        s5_scores(0)
        s5_scores(1)
        s6_amix(0)
        s7_g(0)
        s6_amix(1)
        s8_el(0)
        s7_g(1)
        s8_el(1)

    nc.compile()
    _NC_CACHE["nc"] = nc
    return nc


def _np_reference(H, attention_mask, w_start, b_start, w_q, b_q, w_k, b_k,
                  w_v, b_v, w_cmp, b_cmp):
    NEG = -1e9
    H = H.astype(np.float32)
    pad = attention_mask == 0
    sl = (H @ w_start + b_start)[..., 0]
    sl = np.where(pad, NEG, sl)
    x = sl - sl.max(-1, keepdims=True)
    e = np.exp(x); sp = e / e.sum(-1, keepdims=True)
    idx = np.argsort(-sp, axis=-1, kind="stable")[:, :K]
    tp = np.take_along_axis(sp, idx, axis=1)
    sr = np.take_along_axis(H, idx[..., None], axis=1)
    Q = sr @ w_q + b_q
    K_ = H @ w_k + b_k
    V = H @ w_v + b_v
    sc = np.einsum('bkd,bld->bkl', Q, K_) * SCALE
    sc = np.where(pad[:, None, :], NEG, sc)
    sc = sc - sc.max(-1, keepdims=True)
    a = np.exp(sc); a = a / a.sum(-1, keepdims=True)
    ctx_ = np.einsum('bkl,bld->bkd', a, V)
    tcmp = H @ w_cmp + b_cmp
    es = np.einsum('bkd,bld->bkl', ctx_, tcmp) * SCALE
    es = np.where(pad[:, None, :], NEG, es)
    w = tp / (tp.sum(-1, keepdims=True) + 1e-9)
    el = np.einsum('bk,bkl->bl', w, es)
    el = np.where(pad, NEG, el)
    return sl, el


def kernel(**inputs):
    H = np.asarray(inputs["H"], np.float32)
    mask = np.asarray(inputs["attention_mask"])
    b_start = np.asarray(inputs["b_start"], np.float32)
    biases_zero = all(np.all(np.asarray(inputs[n]) == 0)
                      for n in ["b_q", "b_k", "b_v", "b_cmp"])
    if not bool((mask == 1).all()) or not biases_zero:
        sl, el = _np_reference(**{k: np.asarray(v) for k, v in inputs.items()})
        return np.asarray(sl, np.float32), np.asarray(el, np.float32)

    w_start = np.asarray(inputs["w_start"], np.float32)
    w_q = np.asarray(inputs["w_q"], np.float32)
    w_k = np.asarray(inputs["w_k"], np.float32)
    w_v = np.asarray(inputs["w_v"], np.float32)
    w_cmp = np.asarray(inputs["w_cmp"], np.float32)

    hi = H.astype(bfnp)
    lo = (H - hi.astype(np.float32)).astype(bfnp)
    hthi = np.ascontiguousarray(hi.transpose(0, 2, 1))            # [B, D, L]
    wqk = ((w_q @ w_k.T) * SCALE).astype(bfnp)
    wvc = ((w_v @ w_cmp.T) * SCALE).astype(bfnp)
    wsh = w_start[:, 0].astype(bfnp)
    wsl = (w_start[:, 0] - wsh.astype(np.float32)).astype(bfnp)
    ws = np.stack([wsh, wsl, wsh], axis=-1).reshape(DC, 128, 3)
    offs = np.repeat(np.arange(8, dtype=np.float32) * 512, 8)[None, :]

    nc = _build_nc()
    in_maps = []
    for c in range(NCORES):
        s = slice(c * BPC, (c + 1) * BPC)
        natcat = np.concatenate([hi[s].reshape(BPC * L, D),
                                 lo[s].reshape(BPC * L, D)], axis=0)
        in_maps.append({
            "hthi": hthi[s],
            "natcat": np.ascontiguousarray(natcat),
            "wqk": wqk, "wvc": wvc, "ws": ws,
            "id64": np.eye(64, dtype=bfnp),
            "id64f": np.eye(64, dtype=np.float32),
            "i1": np.ones((1, 1), bfnp), "i1f": np.ones((1, 1), np.float32),
            "offs": offs,
        })
    import time as _time
    _t0 = _time.time()
    res = run_bass_kernel_spmd(nc, in_maps, core_ids=list(range(NCORES)))
    if os.environ.get("KERNEL_TIME"):
        print(f"[kernel] device dispatch+exec wall: {_time.time() - _t0:.3f}s")
    sl = np.concatenate([r["sl"].astype(np.float32) for r in res.results], 0)
    el = np.concatenate([r["el"].astype(np.float32) for r in res.results], 0)
    return (sl + b_start[0]).astype(np.float32), el.astype(np.float32)


# revision 6
# speedup vs baseline: 1.1008x; 1.0203x over previous
import sys, os
sys.path.insert(0, "/opt/trn_rl_repo")
import numpy as np
import ml_dtypes
from contextlib import ExitStack

import concourse.bass as bass
import concourse.bacc as bacc
import concourse.tile as tile
from concourse import mybir
from concourse.bass_utils import run_bass_kernel_spmd

f32 = mybir.dt.float32
bf16 = mybir.dt.bfloat16
u32 = mybir.dt.uint32
AF = mybir.ActivationFunctionType
ALU = mybir.AluOpType
AX = mybir.AxisListType
bfnp = ml_dtypes.bfloat16

B, L, D, K = 16, 4096, 1024, 5
NCORES = 8
BPC = B // NCORES
LC, DC = L // 128, D // 128
NL8 = L // 512
SCALE = 1.0 / float(np.sqrt(D))

_NC_CACHE = {}


def _build_nc():
    if "nc" in _NC_CACHE:
        return _NC_CACHE["nc"]
    nc = bacc.Bacc("TRN2", target_bir_lowering=False, debug=False,
                   num_devices=NCORES)
    dI = lambda n, s, dt=bf16: nc.dram_tensor(n, s, dt, kind="ExternalInput").ap()
    hthi_d = dI("hthi", [BPC, D, L])            # H^T hi, d-major
    natcat_d = dI("natcat", [2 * BPC * L, D])   # [hi rows; lo rows], natural
    wqk_d = dI("wqk", [D, D])                   # (w_q @ w_k.T) * SCALE
    wvc_d = dI("wvc", [D, D])                   # (w_v @ w_cmp.T) * SCALE
    ws_d = dI("ws", [DC, 128, 3])               # [wsh, wsl, wsh]
    id64_d = dI("id64", [64, 64])
    id64f_d = dI("id64f", [64, 64], f32)
    i1_d = dI("i1", [1, 1])
    i1f_d = dI("i1f", [1, 1], f32)
    offs_d = dI("offs", [1, 64], f32)           # offs[0, c*8+j] = 512*c
    scr_d = nc.dram_tensor("scr", [BPC * 64, 1], f32, kind="Internal").ap()
    sl_d = nc.dram_tensor("sl", [BPC, L], bf16, kind="ExternalOutput").ap()
    el_d = nc.dram_tensor("el", [BPC, L], bf16, kind="ExternalOutput").ap()

    NQ = 4                                       # ht quarter tiles
    QL = L // NQ                                 # 1024 l per quarter

    with tile.TileContext(nc) as tc, ExitStack() as ctx:
        res = ctx.enter_context(tc.tile_pool(name="res", bufs=1))
        stg = ctx.enter_context(tc.tile_pool(name="stg", bufs=4))
        sm = ctx.enter_context(tc.tile_pool(name="sm", bufs=1))
        psc = ctx.enter_context(tc.tile_pool(name="psc", bufs=4, space="PSUM"))
        psm = ctx.enter_context(tc.tile_pool(name="psm", bufs=1, space="PSUM"))
        pss = ctx.enter_context(tc.tile_pool(name="pss", bufs=1, space="PSUM"))

        # ---- resident tiles
        wqk_sb = res.tile([128, DC, D], bf16)
        wvc_sb = res.tile([128, DC, D], bf16)
        ws_sb = res.tile([128, DC, 3], bf16)
        id64 = res.tile([64, 64], bf16)
        id64f = res.tile([64, 64], f32)
        i1 = res.tile([1, 1], bf16)
        i1f = res.tile([1, 1], f32)
        offs = res.tile([1, 64], f32)
        ht_sb = [[res.tile([128, DC, QL], bf16, tag=f"ht{b}q{q}",
                           name=f"ht{b}q{q}") for q in range(NQ)]
                 for b in range(BPC)]

        def htc(b, lc8):
            # [128, 512] rhs slice for l-chunk lc8, per dc
            q, off = lc8 * NQ // NL8, (lc8 % (NL8 // NQ)) * 512
            return lambda dc: ht_sb[b][q][:, dc, off:off + 512]

        def load_ht(b, qs=None, eng=None):
            if eng is None:
                eng = nc.sync
            for q in (range(NQ) if qs is None else qs):
                for dc in range(DC):
                    eng.dma_start(
                        ht_sb[b][q][:, dc, :],
                        hthi_d[b, dc * 128:(dc + 1) * 128,
                               q * QL:(q + 1) * QL])

        def load_small_consts():
            for dc in range(DC):
                nc.sync.dma_start(ws_sb[:, dc, :], ws_d[dc])
            nc.sync.dma_start(id64[:], id64_d[:])
            nc.sync.dma_start(id64f[:], id64f_d[:])
            nc.sync.dma_start(i1[:], i1_d[:])
            nc.sync.dma_start(i1f[:], i1f_d[:])
            nc.sync.dma_start(offs[:], offs_d[:])

        def load_weights():
            for dc in range(DC):
                nc.scalar.dma_start(wqk_sb[:, dc, :],
                                    wqk_d[dc * 128:(dc + 1) * 128, :])
            for dc in range(DC):
                nc.sync.dma_start(wvc_sb[:, dc, :],
                                  wvc_d[dc * 128:(dc + 1) * 128, :])

        st = [dict() for _ in range(BPC)]   # per-example cross-phase tiles

        def s1_start_logits(b, chunks=None):
            if chunks is None:
                chunks = range(NL8)
            if "ci64" not in st[b]:
                st[b]["ci64"] = sm.tile([1, 64], f32, tag=f"ci64_{b}",
                                        name=f"ci64_{b}")
            ci64 = st[b]["ci64"]

            for lc8 in chunks:
                rhs = htc(b, lc8)
                pc = psc.tile([64, 512], f32, tag="chunk", name="pc")
                for dc in range(DC):
                    _mm = nc.tensor.matmul(pc[0:1, :], ws_sb[:, dc, 0:1],
                                           rhs(dc), start=(dc == 0),
                                           stop=(dc == DC - 1))
                    if b == 0 and lc8 == 0 and os.environ.get("TILE_FOLLOW"):
                        tile.tile_follow(_mm, log_all_deps=True)
                sls1 = sm.tile([1, 512], bf16, tag="sls1", name="sls1", bufs=2)
                nc.scalar.copy(sls1[:], pc[0:1, :])
                nc.scalar.dma_start(sl_d[b:b + 1, lc8 * 512:(lc8 + 1) * 512],
                                    sls1[:])
                c8v = sm.tile([1, 8], bf16, tag="c8v", name="c8v")
                c8i = sm.tile([1, 8], u32, tag="c8i", name="c8i")
                nc.vector.max(c8v[:], sls1[:])
                nc.vector.max_index(c8i[:], c8v[:], sls1[:])
                nc.vector.tensor_copy(ci64[0:1, lc8 * 8:(lc8 + 1) * 8], c8i[:])

        def s2a_gather(b):
            # non-PE: candidate l list -> [64,1] index tiles -> row gathers
            ci64 = st[b]["ci64"]
            nc.vector.tensor_tensor(ci64[:], ci64[:], offs[:], ALU.add)
            cif_t = sm.tile([64, 1], f32, tag="cif_t", name="cif_t")
            nc.scalar.dma_start(
                scr_d[b * 64:(b + 1) * 64].rearrange("a x -> x a"),
                ci64[0:1, :])
            nc.scalar.dma_start(cif_t[:], scr_d[b * 64:(b + 1) * 64])
            candl = sm.tile([64, 1], u32, tag="candl", name="candl")
            nc.vector.tensor_scalar(candl[:], cif_t[:], float(b * L), None,
                                    ALU.add)
            candlb = sm.tile([64, 1], u32, tag="candlb", name="candlb")
            nc.vector.tensor_scalar(candlb[:], cif_t[:],
                                    float(b * L + BPC * L), None, ALU.add)
            hi64 = sm.tile([64, D], bf16, tag="hi64", name="hi64")
            lo64 = sm.tile([64, D], bf16, tag="lo64", name="lo64")
            st[b]["hi64"], st[b]["lo64"] = hi64, lo64
            nc.gpsimd.indirect_dma_start(
                out=hi64[:], out_offset=None, in_=natcat_d[:],
                in_offset=bass.IndirectOffsetOnAxis(ap=candl[:, 0:1], axis=0))
            nc.gpsimd.indirect_dma_start(
                out=lo64[:], out_offset=None, in_=natcat_d[:],
                in_offset=bass.IndirectOffsetOnAxis(ap=candlb[:, 0:1], axis=0))

        def s2b_refine(b):
            hi64, lo64 = st[b]["hi64"], st[b]["lo64"]
            phi = pss.tile([128, DC, 64], bf16, tag="smb", name="phi")
            for dc in range(DC):
                nc.tensor.transpose(phi[:, dc, :],
                                    hi64[:, dc * 128:(dc + 1) * 128], id64[:])
            hi64T = sm.tile([128, DC, 64], bf16, tag="hi64T", name="hi64T")
            st[b]["hi64T"] = hi64T
            nc.vector.tensor_copy(hi64T[:, 0:4, :], phi[:, 0:4, :])
            nc.vector.tensor_copy(hi64T[:, 4:DC, :], phi[:, 4:DC, :])
            plo = pss.tile([128, DC, 64], bf16, tag="smb", name="plo")
            for dc in range(DC):
                nc.tensor.transpose(plo[:, dc, :],
                                    lo64[:, dc * 128:(dc + 1) * 128], id64[:])
            lo64T = sm.tile([128, DC, 64], bf16, tag="lo64T", name="lo64T")
            nc.scalar.copy(lo64T[:, 0:4, :], plo[:, 0:4, :])
            nc.scalar.copy(lo64T[:, 4:DC, :], plo[:, 4:DC, :])

            pcor = pss.tile([128, 64], f32, tag="smf", name="pcor")
            for dc in range(DC):
                nc.tensor.matmul(pcor[0:64, 0:2], hi64T[:, dc, :],
                                 ws_sb[:, dc, 0:2],
                                 start=(dc == 0), stop=(dc == DC - 1),
                                 skip_group_check=True)
                nc.tensor.matmul(pcor[0:64, 2:3], lo64T[:, dc, :],
                                 ws_sb[:, dc, 2:3],
                                 start=False, stop=(dc == DC - 1),
                                 skip_group_check=True)
            ex64p = sm.tile([64, 1], f32, tag=f"ex64p_{b}", name=f"ex64p_{b}")
            st[b]["ex64p"] = ex64p
            nc.vector.tensor_reduce(ex64p[:], pcor[0:64, 0:3], AX.X, ALU.add)

        def s2c_weights(b):
            ex64p = st[b]["ex64p"]
            pex = pss.tile([128, 64], f32, tag="smf", name="pex")
            nc.tensor.transpose(pex[0:1, 0:64], ex64p[:], id64f[:])
            ex64f = sm.tile([1, 64], f32, tag="ex64f", name="ex64f")
            nc.vector.tensor_copy(ex64f[:], pex[0:1, 0:64])
            top8v = sm.tile([1, 8], f32, tag="top8v", name="top8v")
            nc.vector.max(top8v[:], ex64f[:])
            mask64 = sm.tile([1, 64], f32, tag="mask64", name="mask64")
            nc.vector.tensor_scalar(mask64[:], ex64f[:], top8v[0:1, 4:5],
                                    None, ALU.is_ge)
            e64 = sm.tile([1, 64], f32, tag="e64", name="e64")
            nc.scalar.activation(e64[:], ex64f[:], AF.Exp)
            nc.vector.tensor_tensor(e64[:], e64[:], mask64[:], ALU.mult)
            s64 = sm.tile([1, 1], f32, tag="s64", name="s64")
            nc.vector.tensor_reduce(s64[:], e64[:], AX.X, ALU.add)
            rs64 = sm.tile([1, 1], f32, tag="rs64", name="rs64")
            nc.vector.reciprocal(rs64[:], s64[:])
            w64 = sm.tile([1, 64], f32, tag="w64", name="w64")
            nc.vector.tensor_scalar_mul(w64[:], e64[:], rs64[:])
            pw = pss.tile([128, 64], f32, tag="smf", name="pw")
            nc.tensor.transpose(pw[0:64, 0:1], w64[:], i1f[:])
            w64T = sm.tile([64, 1], f32, tag=f"w64T_{b}", name=f"w64T_{b}")
            st[b]["w64T"] = w64T
            nc.vector.tensor_copy(w64T[:], pw[0:64, 0:1])

        def s4_P(b):
            hi64T = st[b]["hi64T"]
            pP = psm.tile([64, D], f32, tag="mid", name="pP")
            for o in range(2):
                for dci in range(DC):
                    nc.tensor.matmul(pP[:, o * 512:(o + 1) * 512],
                                     hi64T[:, dci, :],
                                     wqk_sb[:, dci, o * 512:(o + 1) * 512],
                                     start=(dci == 0), stop=(dci == DC - 1),
                                     skip_group_check=True)
            P_sb = sm.tile([64, D], bf16, tag="P_sb", name="P_sb")
            nc.scalar.copy(P_sb[:, 0:512], pP[:, 0:512])
            nc.scalar.copy(P_sb[:, 512:D], pP[:, 512:D])
            pPT = pss.tile([128, DC, 64], bf16, tag="smb", name="pPT")
            for dc in range(DC):
                nc.tensor.transpose(pPT[:, dc, :],
                                    P_sb[:, dc * 128:(dc + 1) * 128], id64[:])
            PT = sm.tile([128, DC, 64], bf16, tag=f"PT_{b}", name=f"PT_{b}")
            st[b]["PT"] = PT
            nc.vector.tensor_copy(PT[:], pPT[:])

        def s5_scores(b):
            PT, w64T = st[b]["PT"], st[b]["w64T"]
            E_sb = sm.tile([64, L], bf16, tag="E_sb", name="E_sb")
            z64a = sm.tile([64, NL8], f32, tag="z64a", name="z64a")
            for lc8 in range(NL8):
                rhs = htc(b, lc8)
                psc5 = psc.tile([64, 512], f32, tag="chunk", name="psc5")
                for dc in range(DC):
                    nc.tensor.matmul(psc5[:], PT[:, dc, :], rhs(dc),
                                     start=(dc == 0), stop=(dc == DC - 1))
                nc.scalar.activation(E_sb[:, lc8 * 512:(lc8 + 1) * 512],
                                     psc5[:], AF.Exp,
                                     accum_out=z64a[:, lc8:lc8 + 1])
            z64 = sm.tile([64, 1], f32, tag="z64", name="z64")
            nc.vector.tensor_reduce(z64[:], z64a[:], AX.X, ALU.add)
            rz64 = sm.tile([64, 1], f32, tag="rz64", name="rz64")
            nc.vector.reciprocal(rz64[:], z64[:])
            c64 = sm.tile([64, 1], bf16, tag="c64", name="c64")
            nc.vector.tensor_tensor(c64[:], w64T[:], rz64[:], ALU.mult)
            pm32 = pss.tile([128, 64], f32, tag="smf", name="pm32")
            for c in range(LC):
                nc.tensor.matmul(pm32[:, c:c + 1],
                                 E_sb[:, c * 128:(c + 1) * 128], c64[:],
                                 start=True, stop=True, skip_group_check=True)
            m32 = sm.tile([128, LC], bf16, tag=f"m32_{b}", name=f"m32_{b}")
            st[b]["m32"] = m32
            nc.vector.tensor_copy(m32[:], pm32[:, 0:LC])

        def s6_amix(b):
            m32 = st[b]["m32"]
            pam = psm.tile([64, D], f32, tag="mid", name="pam")
            for lcp in range(LC // 2):
                natc = stg.tile([128, 2, D], bf16, tag="natstg", name="natc")
                nc.sync.dma_start(
                    natc[:],
                    natcat_d[b * L + lcp * 256:b * L + (lcp + 1) * 256, :]
                    .rearrange("(t p) d -> p t d", p=128))
                for t in range(2):
                    lc = lcp * 2 + t
                    for o in range(2):
                        nc.tensor.matmul(pam[0:1, o * 512:(o + 1) * 512],
                                         m32[:, lc:lc + 1],
                                         natc[:, t, o * 512:(o + 1) * 512],
                                         start=(lc == 0), stop=(lc == LC - 1),
                                         skip_group_check=True)
            amix = sm.tile([1, D], bf16, tag="amix", name="amix")
            nc.scalar.copy(amix[0:1, 0:512], pam[0:1, 0:512])
            nc.scalar.copy(amix[0:1, 512:D], pam[0:1, 512:D])
            pamT = pss.tile([128, DC, 64], bf16, tag="smb", name="pamT")
            for dc in range(DC):
                nc.tensor.transpose(pamT[:, dc, 0:1],
                                    amix[0:1, dc * 128:(dc + 1) * 128], i1[:])
            amixT = sm.tile([128, DC, 1], bf16, tag=f"amixT_{b}",
                            name=f"amixT_{b}")
            st[b]["amixT"] = amixT
            nc.vector.tensor_copy(amixT[:], pamT[:, :, 0:1])

        def s7_g(b):
            amixT = st[b]["amixT"]
            pg = psm.tile([64, D], f32, tag="mid", name="pg")
            for o in range(2):
                for dci in range(DC):
                    nc.tensor.matmul(pg[0:1, o * 512:(o + 1) * 512],
                                     amixT[:, dci, :],
                                     wvc_sb[:, dci, o * 512:(o + 1) * 512],
                                     start=(dci == 0), stop=(dci == DC - 1),
                                     skip_group_check=True)
            g_sb = sm.tile([1, D], bf16, tag="g_sb", name="g_sb")
            nc.scalar.copy(g_sb[0:1, 0:512], pg[0:1, 0:512])
            nc.scalar.copy(g_sb[0:1, 512:D], pg[0:1, 512:D])
            pgT = pss.tile([128, DC, 64], bf16, tag="smb", name="pgT")
            for dc in range(DC):
                nc.tensor.transpose(pgT[:, dc, 0:1],
                                    g_sb[0:1, dc * 128:(dc + 1) * 128], i1[:])
            gT = sm.tile([128, DC, 1], bf16, tag=f"gT_{b}", name=f"gT_{b}")
            st[b]["gT"] = gT
            nc.vector.tensor_copy(gT[:], pgT[:, :, 0:1])

        def s8_el(b):
            gT = st[b]["gT"]
            for lc8 in range(NL8):
                rhs = htc(b, lc8)
                pe = psc.tile([64, 512], f32, tag="chunk", name="pe")
                for dc in range(DC):
                    nc.tensor.matmul(pe[0:1, :], gT[:, dc, :], rhs(dc),
                                     start=(dc == 0), stop=(dc == DC - 1))
                els = sm.tile([1, 512], bf16, tag="els", name="els", bufs=2)
                nc.vector.tensor_copy(els[:], pe[0:1, :])
                nc.scalar.dma_start(el_d[b:b + 1, lc8 * 512:(lc8 + 1) * 512],
                                    els[:])

        # ---------------- schedule ----------------
        load_small_consts()
        load_ht(0, [0], eng=nc.scalar)
        load_ht(0, [1])
        s1_start_logits(0, [0, 1])
        load_ht(0, [2])
        s1_start_logits(0, [2, 3])
        load_ht(0, [3])
        s1_start_logits(0, [4, 5])
        load_ht(1, [0])
        s1_start_logits(0, [6, 7])
        s2a_gather(0)
        load_ht(1, [1, 2])
        s1_start_logits(1, [0, 1])
        load_ht(1, [3])
        load_weights()
        s1_start_logits(1, [2, 3, 4, 5, 6, 7])
        s2b_refine(0)
        s2a_gather(1)
        s4_P(0)
        s2c_weights(0)
        s2b_refine(1)
        s4_P(1)
        s2c_weights(1)
        s5_scores(0)
        s5_scores(1)
        s6_amix(0)
        s7_g(0)
        s6_amix(1)
        s8_el(0)
        s7_g(1)
        s8_el(1)

    nc.compile()
    _NC_CACHE["nc"] = nc
    return nc


def _np_reference(H, attention_mask, w_start, b_start, w_q, b_q, w_k, b_k,
                  w_v, b_v, w_cmp, b_cmp):
    NEG = -1e9
    H = H.astype(np.float32)
    pad = attention_mask == 0
    sl = (H @ w_start + b_start)[..., 0]
    sl = np.where(pad, NEG, sl)
    x = sl - sl.max(-1, keepdims=True)
    e = np.exp(x); sp = e / e.sum(-1, keepdims=True)
    idx = np.argsort(-sp, axis=-1, kind="stable")[:, :K]
    tp = np.take_along_axis(sp, idx, axis=1)
    sr = np.take_along_axis(H, idx[..., None], axis=1)
    Q = sr @ w_q + b_q
    K_ = H @ w_k + b_k
    V = H @ w_v + b_v
    sc = np.einsum('bkd,bld->bkl', Q, K_) * SCALE
    sc = np.where(pad[:, None, :], NEG, sc)
    sc = sc - sc.max(-1, keepdims=True)
    a = np.exp(sc); a = a / a.sum(-1, keepdims=True)
    ctx_ = np.einsum('bkl,bld->bkd', a, V)
    tcmp = H @ w_cmp + b_cmp
    es = np.einsum('bkd,bld->bkl', ctx_, tcmp) * SCALE
    es = np.where(pad[:, None, :], NEG, es)
    w = tp / (tp.sum(-1, keepdims=True) + 1e-9)
    el = np.einsum('bk,bkl->bl', w, es)
    el = np.where(pad, NEG, el)
    return sl, el


def kernel(**inputs):
    H = np.asarray(inputs["H"], np.float32)
    mask = np.asarray(inputs["attention_mask"])
    b_start = np.asarray(inputs["b_start"], np.float32)
    biases_zero = all(np.all(np.asarray(inputs[n]) == 0)
                      for n in ["b_q", "b_k", "b_v", "b_cmp"])
    if not bool((mask == 1).all()) or not biases_zero:
        sl, el = _np_reference(**{k: np.asarray(v) for k, v in inputs.items()})
        return np.asarray(sl, np.float32), np.asarray(el, np.float32)

    w_start = np.asarray(inputs["w_start"], np.float32)
    w_q = np.asarray(inputs["w_q"], np.float32)
    w_k = np.asarray(inputs["w_k"], np.float32)
    w_v = np.asarray(inputs["w_v"], np.float32)
    w_cmp = np.asarray(inputs["w_cmp"], np.float32)

    hi = H.astype(bfnp)
    lo = (H - hi.astype(np.float32)).astype(bfnp)
    hthi = np.ascontiguousarray(hi.transpose(0, 2, 1))            # [B, D, L]
    wqk = ((w_q @ w_k.T) * SCALE).astype(bfnp)
    wvc = ((w_v @ w_cmp.T) * SCALE).astype(bfnp)
    wsh = w_start[:, 0].astype(bfnp)
    wsl = (w_start[:, 0] - wsh.astype(np.float32)).astype(bfnp)
    ws = np.stack([wsh, wsl, wsh], axis=-1).reshape(DC, 128, 3)
    offs = np.repeat(np.arange(8, dtype=np.float32) * 512, 8)[None, :]

    nc = _build_nc()
    in_maps = []
    for c in range(NCORES):
        s = slice(c * BPC, (c + 1) * BPC)
        natcat = np.concatenate([hi[s].reshape(BPC * L, D),
                                 lo[s].reshape(BPC * L, D)], axis=0)
        in_maps.append({
            "hthi": hthi[s],
            "natcat": np.ascontiguousarray(natcat),
            "wqk": wqk, "wvc": wvc, "ws": ws,
            "id64": np.eye(64, dtype=bfnp),
            "id64f": np.eye(64, dtype=np.float32),
            "i1": np.ones((1, 1), bfnp), "i1f": np.ones((1, 1), np.float32),
            "offs": offs,
        })
    import time as _time
    _t0 = _time.time()
    res = run_bass_kernel_spmd(nc, in_maps, core_ids=list(range(NCORES)))
    if os.environ.get("KERNEL_TIME"):
        print(f"[kernel] device dispatch+exec wall: {_time.time() - _t0:.3f}s")
    sl = np.concatenate([r["sl"].astype(np.float32) for r in res.results], 0)
    el = np.concatenate([r["el"].astype(np.float32) for r in res.results], 0)
    return (sl + b_start[0]).astype(np.float32), el.astype(np.float32)
